# revision 1
# baseline (speedup 1.0000x reference)
"""GCN encoder (2x GCNConv + BatchNorm + PReLU) on 8 Trainium2 NeuronCores.

v2: scatter-free design.
  - nodes sharded contiguously across 8 cores (12500 real + pad -> 12544);
  - per layer: v' = dinv * (h @ W) computed locally, AllGather of the bf16
    v' table (halo exchange);
  - edges dst-sharded (owner of dst), grouped by (dst tile, src quarter),
    padded per group to 128-edge chunks; tiles processed in rounds of RT so
    PSUM accumulators stay bounded;
  - messages fetched with gpsimd.dma_gather from the src quarter of the
    table (quarter-local int16 idxs), one call per (round, quarter);
  - segment-sum on the tensor engine: per chunk a host-built fp8 one-hot
    matrix S[i, j] = (dst_local[i] == j), streamed per round, used as the
    PSUM-accumulated matmul acc[j, d] += S^T @ msg;
  - conv = dinv[dst] * acc + dinv^2 * v (self-loop analytic), transpose
    into actT [feat, rows]; BN stats via free-dim reduce, stats AllReduce,
    fused BN+PReLU via two ACT Relu passes + one DVE scalar_tensor_tensor.

norm_e = dinv[src]*dinv[dst] is separable: dinv[src] folded into the
gathered table, dinv[dst] at read-out.  BatchNorm cancels the conv bias,
so b0/b1 are accepted and ignored.
"""

import numpy as np

import concourse.bass as bass
import concourse.bacc as bacc
import concourse.tile as tile
from concourse import mybir
from concourse import bass_utils
from concourse.masks import make_identity
from concourse.bass_interp import get_hw_module

F32 = mybir.dt.float32
BF16 = mybir.dt.bfloat16
I16 = mybir.dt.int16
EPS = 1e-5
NB = 8            # cores
NQ = 4            # src quarters (int16 idx limit)
RT = 5            # dst tiles per round (PSUM bank budget)


def _wrap16(vals: np.ndarray, cap: int) -> np.ndarray:
    assert vals.shape[0] == cap and cap % 16 == 0
    return np.ascontiguousarray(vals.reshape(cap // 16, 16).T)


def preprocess(x: np.ndarray, edge_index: np.ndarray):
    """Group edges by (dst core, dst tile, src quarter); pad each group to a
    multiple of 128; lay out gather idx streams round-major/quarter-major and
    emit the chunk schedule."""
    N = x.shape[0]
    nsh = (N + NB - 1) // NB                   # 12500 real rows/shard
    SH = ((nsh + 127) // 128) * 128            # 12544 padded
    NTL = SH // 128                            # 98 local tiles
    NR = (NTL + RT - 1) // RT                  # 14 rounds
    QR = (NB * SH) // NQ                       # 25088 rows per src quarter

    src = edge_index[0].astype(np.int64)
    dst = edge_index[1].astype(np.int64)
    deg = np.bincount(dst, minlength=N) + 1.0  # +1 self-loop
    dinv = (1.0 / np.sqrt(deg)).astype(np.float32)

    gsrc = (src // nsh) * SH + (src % nsh)     # padded-global row id
    c_of = dst // nsh
    dloc = dst % nsh
    q_of = gsrc // QR
    t_of = dloc // 128
    r_of = t_of // RT

    # stream order: (core, round, quarter, tile)
    key = ((c_of * NR + r_of) * NQ + q_of) * NTL + t_of
    order = np.argsort(key, kind="stable")
    ks = key[order]
    gs = (gsrc % QR)[order].astype(np.int16)
    dl = (dloc - t_of * 128)[order].astype(np.int32)

    nk = NB * NR * NQ * NTL
    counts = np.bincount(ks, minlength=nk)
    pad = ((counts + 127) // 128) * 128
    # same static shape on all cores: per (r,q,t) max over cores
    pad = pad.reshape(NB, NR * NQ * NTL).max(axis=0)
    pad = np.broadcast_to(pad[None, :], (NB, NR * NQ * NTL)).reshape(-1)

    coff = np.zeros(nk + 1, np.int64)
    np.cumsum(counts, out=coff[1:])
    poff = np.zeros(nk + 1, np.int64)
    np.cumsum(pad, out=poff[1:])
    tot = int(poff[(NR * NQ * NTL)])           # padded slots per core
    nchunks = tot // 128

    gidx = np.zeros((NB, tot), np.int16)
    within = np.arange(len(ks)) - coff[ks]
    slot = poff[ks] + within
    core_of = ks // (NR * NQ * NTL)
    lslot = slot - core_of * tot
    gidx[core_of, lslot] = gs
    # host-built one-hot matrices: oneh[c, slot%128, chunk*128 + dst_local]
    import ml_dtypes
    oneh = np.zeros((NB, 128, nchunks * 128), np.float32)
    oneh[core_of, lslot % 128, (lslot // 128) * 128 + dl] = 1.0
    oneh = oneh.astype(ml_dtypes.float8_e4m3)

    # schedule (identical for all cores): per (r,q,t) chunk counts
    nch = (pad.reshape(NB, NR, NQ, NTL) // 128)[0]   # [NR, NQ, NTL]
    # call sizes per (r, q) in slots; stream offsets
    call_sz = nch.sum(axis=2) * 128                  # [NR, NQ]
    # per (r,q,t): offset of the group inside its call (in chunks)
    grp_off = np.zeros((NR, NQ, NTL), np.int64)
    for r in range(NR):
        for q in range(NQ):
            grp_off[r, q] = np.concatenate([[0], np.cumsum(nch[r, q])[:-1]])
    tile_chunks = nch.sum(axis=1)                    # [NR, NTL]

    gidx_w = np.zeros((NB, 128, tot // 16), np.int16)
    for c in range(NB):
        gidx_w[c] = np.tile(_wrap16(gidx[c], tot), (8, 1))

    dinv_cols = np.zeros((NB, 128, NTL), np.float32)
    x_sh = np.zeros((NB, SH, x.shape[1]), np.float32)
    for c in range(NB):
        lo, hi = c * nsh, min((c + 1) * nsh, N)
        d = np.zeros(SH, np.float32)
        d[: hi - lo] = dinv[lo:hi]
        dinv_cols[c] = d.reshape(NTL, 128).T
        x_sh[c, : hi - lo] = x[lo:hi]

    return dict(
        N=N, nsh=nsh, SH=SH, NTL=NTL, NR=NR, QR=QR, tot=tot, nchunks=nchunks,
        nch=tuple(map(tuple, (tuple(map(tuple, nch[r])) for r in range(NR)))),
        call_sz=tuple(map(tuple, call_sz.tolist())),
        grp_off=tuple(tuple(tuple(int(v) for v in grp_off[r, q])
                            for q in range(NQ)) for r in range(NR)),
        tile_chunks=tuple(map(tuple, tile_chunks.tolist())),
        gidx=gidx_w, oneh=oneh, dinv_cols=dinv_cols, x_sh=x_sh,
    )


# -------------------------------------------------------------- device side


def build_kernel(N, SH, NTL, NR, QR, tot, nchunks, nch, call_sz, grp_off,
                 tile_chunks, D=128):
    nc = bacc.Bacc("TRN2", target_bir_lowering=False, debug=False,
                   num_devices=NB)
    rg = [list(range(NB))]
    chunks512 = [(o, min(512, SH - o)) for o in range(0, SH, 512)]
    MTC = max(max(row) for row in call_sz)         # max slots per call
    RCH = max(sum(row) for row in call_sz) // 128  # max chunks per round

    x_in = nc.dram_tensor("x", [SH, D], F32, kind="ExternalInput")
    gidx_in = nc.dram_tensor("gidx", [128, tot // 16], I16, kind="ExternalInput")
    oneh_in = nc.dram_tensor("oneh", [128, nchunks * 128], mybir.dt.float8e4,
                             kind="ExternalInput")
    dinv_in = nc.dram_tensor("dinv_cols", [128, NTL], F32, kind="ExternalInput")
    w_in = [nc.dram_tensor(f"w{l}", [D, D], F32, kind="ExternalInput")
            for l in range(2)]
    gam_in = [nc.dram_tensor(f"gamma{l}", [D, 1], F32, kind="ExternalInput")
              for l in range(2)]
    bet_in = [nc.dram_tensor(f"beta{l}", [D, 1], F32, kind="ExternalInput")
              for l in range(2)]
    a_in = [nc.dram_tensor(f"a{l}", [D, 1], F32, kind="ExternalInput")
            for l in range(2)]
    out_t = nc.dram_tensor("out", [SH, D], F32, kind="ExternalOutput")

    vloc = nc.dram_tensor("vloc", [SH, D], BF16)
    vfull = nc.dram_tensor("vfull", [NB * SH, D], BF16, addr_space="Shared")
    stats_in = nc.dram_tensor("stats_in", [D, 2], F32)
    stats_out = nc.dram_tensor("stats_out", [D, 2], F32, addr_space="Shared")

    out_r = out_t.ap().rearrange("(t p) f -> t p f", p=128)
    x_r = x_in.ap().rearrange("(t p) f -> t p f", p=128)
    vloc_r = vloc.ap().rearrange("(t p) f -> t p f", p=128)

    with tile.TileContext(nc) as tc:
        with (
            tc.tile_pool(name="pers", bufs=1) as PE_,
            tc.tile_pool(name="act", bufs=1) as PA,
            tc.tile_pool(name="msg", bufs=2) as PM,
            tc.tile_pool(name="work", bufs=3) as PW,
            tc.tile_pool(name="small", bufs=2) as PS,
            tc.tile_pool(name="psV", bufs=1, space="PSUM") as PV,
            tc.tile_pool(name="psA", bufs=5, space="PSUM") as PP,
            tc.tile_pool(name="psT", bufs=2, space="PSUM") as PT,
        ):
            ident = PE_.tile([128, 128], F32, tag="ident")
            make_identity(nc, ident[:])
            gidx_sb = PE_.tile([128, tot // 16], I16, tag="gidx")
            nc.sync.dma_start(gidx_sb[:], gidx_in.ap())

            dinv_sb = PE_.tile([128, NTL], F32, tag="dinv")
            nc.sync.dma_start(dinv_sb[:], dinv_in.ap())
            w_sb, gam_sb, bet_sb, a_sb = [], [], [], []
            for l in range(2):
                w_sb.append(PE_.tile([128, 128], F32, tag=f"w{l}", name=f"w{l}_sb"))
                nc.sync.dma_start(w_sb[l][:], w_in[l].ap())
                gam_sb.append(PE_.tile([128, 1], F32, tag=f"g{l}", name=f"g{l}_sb"))
                nc.sync.dma_start(gam_sb[l][:], gam_in[l].ap())
                bet_sb.append(PE_.tile([128, 1], F32, tag=f"b{l}", name=f"b{l}_sb"))
                nc.sync.dma_start(bet_sb[l][:], bet_in[l].ap())
                a_sb.append(PE_.tile([128, 1], F32, tag=f"a{l}", name=f"a{l}_sb"))
                nc.sync.dma_start(a_sb[l][:], a_in[l].ap())
            zero_sb = PE_.tile([128, 128], F32, tag="zero")
            nc.vector.memset(zero_sb[:], 0.0)
            eps_sb = PE_.tile([128, 1], F32, tag="eps")
            nc.vector.memset(eps_sb[:], EPS)

            actT = PA.tile([128, SH], F32, tag="actT")        # h [feat, rows]
            vself = PA.tile([128, NTL, 128], BF16, tag="vself")  # dinv^2*v rows

            # ---- load x, transpose into actT
            for t in range(NTL):
                xt = PW.tile([128, 128], F32, tag="xt")
                nc.sync.dma_start(xt[:], x_r[t])
                tp = PT.tile([128, 128], F32, tag="tp")
                nc.tensor.transpose(out=tp[:], in_=xt[:], identity=ident[:])
                nc.vector.tensor_copy(actT[:, 128 * t : 128 * (t + 1)], tp[:])

            for l in range(2):
                # ---- v' = dinv * (h @ W) -> bf16 vloc + bf16 self term
                for (o, cw) in chunks512:
                    vp = PV.tile([128, 512], F32, tag="vp")
                    nc.tensor.matmul(out=vp[:, :cw], lhsT=w_sb[l][:],
                                     rhs=actT[:, o : o + cw],
                                     start=True, stop=True)
                    vt = PW.tile([128, 512], F32, tag="vt")
                    nc.vector.tensor_copy(vt[:, :cw], vp[:, :cw])
                    for s in range(0, cw, 128):
                        t = (o + s) // 128
                        tp = PT.tile([128, 128], F32, tag="tp")
                        nc.tensor.transpose(out=tp[:], in_=vt[:, s : s + 128],
                                            identity=ident[:])
                        vv = PW.tile([128, 128], BF16, tag="vv")
                        nc.vector.tensor_scalar(
                            vv[:], tp[:], dinv_sb[:, t : t + 1], None,
                            op0=mybir.AluOpType.mult)
                        nc.sync.dma_start(vloc_r[t], vv[:])
                        nc.vector.tensor_scalar(
                            vself[:, t], vv[:], dinv_sb[:, t : t + 1], None,
                            op0=mybir.AluOpType.mult)

                # ---- halo exchange
                nc.gpsimd.collective_compute(
                    "AllGather", mybir.AluOpType.bypass, replica_groups=rg,
                    ins=[vloc.ap().opt()], outs=[vfull.ap().opt()])

                # ---- gather + one-hot matmul segment sum, rounds of RT tiles
                soff = 0          # stream slot offset
                ci = 0            # stream chunk index
                for r in range(NR):
                    t0 = r * RT
                    tiles = list(range(t0, min(t0 + RT, NTL)))
                    rch = sum(call_sz[r]) // 128   # chunks this round
                    ci0 = ci
                    ohl = PM.tile([128, RCH, 128], mybir.dt.float8e4,
                                  tag="ohl")
                    if rch:
                        nc.sync.dma_start(
                            ohl[:, :rch, :],
                            oneh_in.ap()[:, ci0 * 128 : (ci0 + rch) * 128])
                    mts = []
                    callbase = []
                    for q in range(NQ):
                        sz = call_sz[r][q]
                        callbase.append(ci)
                        if sz == 0:
                            mts.append(None)
                            continue
                        mt = PM.tile([128, MTC // 128, 128], BF16,
                                     tag=f"mt{q}")
                        nc.gpsimd.dma_gather(
                            out_ap=mt[:, : sz // 128, :],
                            in_ap=vfull.ap()[q * QR : (q + 1) * QR, :],
                            idxs_ap=gidx_sb[:, soff // 16 : (soff + sz) // 16],
                            num_idxs=sz, num_idxs_reg=sz, elem_size=D,
                            single_packet=False)
                        mts.append(mt)
                        soff += sz
                        ci += sz // 128
                    for t in tiles:
                        total = tile_chunks[r][t]
                        if total == 0:
                            sc = PW.tile([128, 128], F32, tag="sc")
                            nc.vector.tensor_copy(sc[:], vself[:, t])
                        else:
                            acc = PP.tile([128, 128], F32, tag="acc")
                            done = 0
                            for q in range(NQ):
                                nchq = nch[r][q][t]
                                for k in range(nchq):
                                    cik = callbase[q] + grp_off[r][q][t] + k
                                    nc.tensor.matmul(
                                        out=acc[:], lhsT=ohl[:, cik - ci0, :],
                                        rhs=mts[q][:, grp_off[r][q][t] + k, :],
                                        start=(done == 0),
                                        stop=(done == total - 1))
                                    done += 1
                            sc = PW.tile([128, 128], F32, tag="sc")
                            nc.vector.scalar_tensor_tensor(
                                out=sc[:], in0=acc[:],
                                scalar=dinv_sb[:, t : t + 1],
                                in1=vself[:, t],
                                op0=mybir.AluOpType.mult,
                                op1=mybir.AluOpType.add)
                        tp = PT.tile([128, 128], F32, tag="tp")
                        nc.tensor.transpose(out=tp[:], in_=sc[:],
                                            identity=ident[:])
                        nc.scalar.activation(
                            out=actT[:, 128 * t : 128 * (t + 1)], in_=tp[:],
                            func=mybir.ActivationFunctionType.Copy)

                # ---- BN stats (biased over real N rows; pad rows are 0)
                nk = len(chunks512)
                sumc = PS.tile([128, nk], F32, tag="sumc")
                sqc = PS.tile([128, nk], F32, tag="sqc")
                for k, (o, cw) in enumerate(chunks512):
                    nc.vector.tensor_reduce(
                        out=sumc[:, k : k + 1], in_=actT[:, o : o + cw],
                        axis=mybir.AxisListType.X, op=mybir.AluOpType.add)
                    sq = PW.tile([128, 512], F32, tag="sq")
                    nc.scalar.activation(
                        out=sq[:, :cw], in_=actT[:, o : o + cw],
                        func=mybir.ActivationFunctionType.Square,
                        bias=zero_sb[:, 0:1],
                        accum_out=sqc[:, k : k + 1])
                stats_sb = PS.tile([128, 2], F32, tag="stats")
                nc.vector.tensor_reduce(out=stats_sb[:, 0:1], in_=sumc[:],
                                        axis=mybir.AxisListType.X,
                                        op=mybir.AluOpType.add)
                nc.vector.tensor_reduce(out=stats_sb[:, 1:2], in_=sqc[:],
                                        axis=mybir.AxisListType.X,
                                        op=mybir.AluOpType.add)
                nc.sync.dma_start(stats_in.ap(), stats_sb[:])
                nc.gpsimd.collective_compute(
                    "AllReduce", mybir.AluOpType.add, replica_groups=rg,
                    ins=[stats_in.ap().opt()], outs=[stats_out.ap().opt()])
                stats2 = PS.tile([128, 2], F32, tag="stats2")
                nc.sync.dma_start(stats2[:], stats_out.ap())

                # ---- BN affine params
                mu = PS.tile([128, 1], F32, tag="mu")
                nc.vector.tensor_scalar(mu[:], stats2[:, 0:1], 1.0 / N, None,
                                        op0=mybir.AluOpType.mult)
                e2 = PS.tile([128, 1], F32, tag="e2")
                nc.vector.tensor_scalar(e2[:], stats2[:, 1:2], 1.0 / N, None,
                                        op0=mybir.AluOpType.mult)
                var = PS.tile([128, 1], F32, tag="var")
                nc.vector.scalar_tensor_tensor(
                    out=var[:], in0=mu[:], scalar=-1.0, in1=mu[:],
                    op0=mybir.AluOpType.mult, op1=mybir.AluOpType.mult)
                nc.vector.tensor_tensor(out=var[:], in0=e2[:], in1=var[:],
                                        op=mybir.AluOpType.add)
                sd = PS.tile([128, 1], F32, tag="sd")
                nc.scalar.activation(out=sd[:], in_=var[:],
                                     func=mybir.ActivationFunctionType.Sqrt,
                                     bias=eps_sb[:, 0:1])
                rinv = PS.tile([128, 1], F32, tag="rinv")
                nc.vector.reciprocal(rinv[:], sd[:])
                alpha = PS.tile([128, 1], F32, tag="alpha")
                nc.vector.tensor_tensor(out=alpha[:], in0=gam_sb[l][:],
                                        in1=rinv[:], op=mybir.AluOpType.mult)
                bias_p = PS.tile([128, 1], F32, tag="biasp")
                nc.vector.scalar_tensor_tensor(
                    out=bias_p[:], in0=alpha[:], scalar=-1.0, in1=mu[:],
                    op0=mybir.AluOpType.mult, op1=mybir.AluOpType.mult)
                nc.vector.tensor_tensor(out=bias_p[:], in0=bet_sb[l][:],
                                        in1=bias_p[:], op=mybir.AluOpType.add)
                nalpha = PS.tile([128, 1], F32, tag="nalpha")
                nc.vector.tensor_scalar(nalpha[:], alpha[:], -1.0, None,
                                        op0=mybir.AluOpType.mult)
                nbias = PS.tile([128, 1], F32, tag="nbias")
                nc.vector.tensor_scalar(nbias[:], bias_p[:], -1.0, None,
                                        op0=mybir.AluOpType.mult)
                na = PS.tile([128, 1], F32, tag="na")
                nc.vector.tensor_scalar(na[:], a_sb[l][:], -1.0, None,
                                        op0=mybir.AluOpType.mult)

                # ---- fused BN + PReLU: y = relu(z) - a*relu(-z)
                for (o, cw) in chunks512:
                    pos = PW.tile([128, 512], F32, tag="pos")
                    nc.scalar.activation(
                        out=pos[:, :cw], in_=actT[:, o : o + cw],
                        func=mybir.ActivationFunctionType.Relu,
                        bias=bias_p[:, :1], scale=alpha[:, :1])
                    neg = PW.tile([128, 512], F32, tag="neg")
                    nc.scalar.activation(
                        out=neg[:, :cw], in_=actT[:, o : o + cw],
                        func=mybir.ActivationFunctionType.Relu,
                        bias=nbias[:, :1], scale=nalpha[:, :1])
                    nc.vector.scalar_tensor_tensor(
                        out=actT[:, o : o + cw], in0=neg[:, :cw],
                        scalar=na[:, :1], in1=pos[:, :cw],
                        op0=mybir.AluOpType.mult, op1=mybir.AluOpType.add)

            # ---- write h2 back as [rows, feat]
            for t in range(NTL):
                tp = PT.tile([128, 128], F32, tag="tp")
                nc.tensor.transpose(out=tp[:],
                                    in_=actT[:, 128 * t : 128 * (t + 1)],
                                    identity=ident[:])
                ot = PW.tile([128, 128], F32, tag="ot")
                nc.vector.tensor_copy(ot[:], tp[:])
                nc.sync.dma_start(out_r[t], ot[:])

    nc.compile()
    return nc


# ------------------------------------------------------------------- driver

_CACHE: dict = {}


def _get_compiled(pre):
    key = (pre["N"], pre["SH"], pre["tot"], pre["call_sz"], pre["nch"],
           pre["tile_chunks"])
    if key not in _CACHE:
        import os
        nc = build_kernel(pre["N"], pre["SH"], pre["NTL"], pre["NR"],
                          pre["QR"], pre["tot"], pre["nchunks"], pre["nch"],
                          pre["call_sz"], pre["grp_off"], pre["tile_chunks"])
        if not os.environ.get("KN_SIM"):
            nc.m = get_hw_module(nc.m)
        _CACHE[key] = nc
    return _CACHE[key]


def make_in_maps(pre, w0, gamma0, beta0, a0, w1, gamma1, beta1, a1):
    def col(v):
        return np.ascontiguousarray(np.asarray(v, np.float32).reshape(-1, 1))

    def rep(v):
        return np.full((128, 1), np.float32(np.asarray(v).reshape(-1)[0]),
                       np.float32)

    maps = []
    for c in range(NB):
        maps.append({
            "x": pre["x_sh"][c],
            "gidx": pre["gidx"][c],
            "oneh": pre["oneh"][c],
            "dinv_cols": pre["dinv_cols"][c],
            "w0": np.ascontiguousarray(np.asarray(w0, np.float32)),
            "w1": np.ascontiguousarray(np.asarray(w1, np.float32)),
            "gamma0": col(gamma0), "beta0": col(beta0), "a0": rep(a0),
            "gamma1": col(gamma1), "beta1": col(beta1), "a1": rep(a1),
        })
    return maps


def kernel(x, edge_index, w0, b0, gamma0, beta0, a0,
           w1, b1, gamma1, beta1, a1, _trace=False):
    x = np.asarray(x, np.float32)
    edge_index = np.asarray(edge_index, np.int64)
    pre = preprocess(x, edge_index)
    nc = _get_compiled(pre)
    in_maps = make_in_maps(pre, w0, gamma0, beta0, a0, w1, gamma1, beta1, a1)
    res = bass_utils.run_bass_kernel_spmd(
        nc, in_maps, core_ids=list(range(NB)), trace=_trace)
    nsh, N = pre["nsh"], pre["N"]
    out = np.concatenate([res.results[c]["out"][:nsh] for c in range(NB)],
                         axis=0)[:N]
    if _trace:
        kernel.last_results = res
    return np.ascontiguousarray(out)



# revision 2
# speedup vs baseline: 2.6642x; 2.6642x over previous
"""GCN encoder (2x GCNConv + BatchNorm + PReLU) on 8 Trainium2 NeuronCores.

v3: scatter-free design, unpadded tile-impure gather chunks, 4 SWDGE queues.
  - nodes sharded contiguously across 8 cores (12500 real + pad -> 12544);
  - per layer: v' = dinv * (h @ W) computed locally, AllGather of the bf16
    v' table (halo exchange);
  - edges dst-sharded (owner of dst), stream sorted by (round, src quarter,
    dst tile); per (round, quarter) ONE gather call (queue_num=q) with only
    trailing padding -> ~107k gather descriptors/layer vs 150k in v2;
  - chunks of 128 edge slots may span tiles: per (chunk, tile) a host-built
    fp8 one-hot block S[slot, j] = (edge slot -> tile row j), streamed per
    round, used as PSUM-accumulated matmuls acc_t[j, d] += S^T @ msg;
  - conv = dinv[dst] * acc + dinv^2 * v (self-loop analytic); BN stats via
    free-dim reduce + AllReduce; fused BN+PReLU via two ACT Relu passes and
    one DVE scalar_tensor_tensor.

norm_e = dinv[src]*dinv[dst] is separable: dinv[src] folded into the
gathered table, dinv[dst] at read-out.  BatchNorm cancels the conv bias,
so b0/b1 are accepted and ignored.
"""

import numpy as np

import concourse.bass as bass
import concourse.bacc as bacc
import concourse.tile as tile
from concourse import mybir
from concourse import bass_utils
from concourse.masks import make_identity
from concourse.bass_interp import get_hw_module

F32 = mybir.dt.float32
BF16 = mybir.dt.bfloat16
I16 = mybir.dt.int16
EPS = 1e-5
NB = 8            # cores
NQ = 4            # src quarters (int16 idx limit)
RT = 7            # dst tiles per round (PSUM budget)


def _wrap16(vals: np.ndarray, cap: int) -> np.ndarray:
    assert vals.shape[0] == cap and cap % 16 == 0
    return np.ascontiguousarray(vals.reshape(cap // 16, 16).T)


def preprocess(x: np.ndarray, edge_index: np.ndarray):
    """Sort edges by (core, round, quarter, tile); pad per (round, quarter)
    call only at the tail; build per-(chunk, tile) one-hot blocks with
    cross-core-max chunk spans and the matmul schedule."""
    N = x.shape[0]
    nsh = (N + NB - 1) // NB                   # 12500 real rows/shard
    SH = ((nsh + 127) // 128) * 128            # 12544 padded
    NTL = SH // 128                            # 98 local tiles
    NR = (NTL + RT - 1) // RT                  # rounds
    QR = (NB * SH) // NQ                       # rows per src quarter

    src = edge_index[0].astype(np.int64)
    dst = edge_index[1].astype(np.int64)
    deg = np.bincount(dst, minlength=N) + 1.0  # +1 self-loop
    dinv = (1.0 / np.sqrt(deg)).astype(np.float32)

    gsrc = (src // nsh) * SH + (src % nsh)     # padded-global row id
    c_of = dst // nsh
    dloc = dst % nsh
    q_of = gsrc // QR
    t_of = dloc // 128
    j_of = dloc % 128
    r_of = t_of // RT

    # edge stream order: (core, round, quarter, tile)
    key = ((c_of * NR + r_of) * NQ + q_of) * NTL + t_of
    order = np.argsort(key, kind="stable")
    ks = key[order]
    gq = (gsrc % QR)[order].astype(np.int16)
    jj = j_of[order].astype(np.int64)

    # per (c, r, q) counts -> shared static call sizes (cross-core max)
    crq = ks // NTL                            # (c*NR + r)*NQ + q
    cnt_crq = np.bincount(crq, minlength=NB * NR * NQ).reshape(NB, NR, NQ)
    P = (np.ceil(cnt_crq.max(axis=0) / 128).astype(np.int64)) * 128  # [NR,NQ]
    P = np.maximum(P, 128)
    call_off = np.zeros(NR * NQ + 1, np.int64)
    np.cumsum(P.reshape(-1), out=call_off[1:])
    tot = int(call_off[-1])                    # padded slots per core / layer

    # slot of each edge inside its (c,r,q) call
    crq_start = np.zeros(NB * NR * NQ + 1, np.int64)
    np.cumsum(cnt_crq.reshape(-1), out=crq_start[1:])
    slot = np.arange(len(ks)) - crq_start[crq]
    chunk = slot // 128

    # gather idx stream (pad idx 0)
    gidx = np.zeros((NB, tot), np.int16)
    c_e = crq // (NR * NQ)
    rq_e = crq % (NR * NQ)
    gidx[c_e, call_off[rq_e] + slot] = gq

    # per-(c,r,q,t) slot ranges -> global chunk spans per (r,q,t)
    uk, ui, uc = np.unique(ks, return_index=True, return_counts=True)
    a_c = slot[ui]
    b_c = a_c + uc
    rqt_u = (uk % (NR * NQ)) * NTL + (uk % NTL)
    # careful: rqt index = ((r*NQ)+q)*NTL + t; uk = ((c*NR+r)*NQ+q)*NTL+t
    rq_u = (uk // NTL) % (NR * NQ)
    t_u = uk % NTL
    rqt_u = rq_u * NTL + t_u
    NG = NR * NQ * NTL
    lo_g = np.full(NG, 2**31, np.int64)
    hi_g = np.zeros(NG, np.int64)
    np.minimum.at(lo_g, rqt_u, a_c // 128)
    np.maximum.at(hi_g, rqt_u, (b_c + 127) // 128)

    # block list ordered (r, t, q, k); schedule per (r, t)
    rb_start = np.zeros(NG, np.int64)          # block index within round
    round_nblk = np.zeros(NR, np.int64)
    round_blk0 = np.zeros(NR + 1, np.int64)
    sched = []                                 # sched[r][t-rel] = [(q,k,rb)..]
    for r in range(NR):
        tiles = range(r * RT, min((r + 1) * RT, NTL))
        rb = 0
        srt = []
        for t in tiles:
            lst = []
            for q in range(NQ):
                g = (r * NQ + q) * NTL + t
                if hi_g[g] > lo_g[g]:
                    rb_start[g] = rb
                    for k in range(int(lo_g[g]), int(hi_g[g])):
                        lst.append((q, k, rb))
                        rb += 1
            srt.append(tuple(lst))
        sched.append(tuple(srt))
        round_nblk[r] = rb
        round_blk0[r + 1] = round_blk0[r] + rb
    nblocks = int(round_blk0[-1])
    NBM = int(round_nblk.max())

    # one-hot blocks, fp8: oneh[c, slot%128, blk*128 + j]
    import ml_dtypes
    g_e = rq_e * NTL + t_of[order]
    blk_e = round_blk0[rq_e // NQ] + rb_start[g_e] + (chunk - lo_g[g_e])
    oneh = np.zeros((NB, 128, nblocks * 128), np.float32)
    oneh[c_e, slot % 128, blk_e * 128 + jj] = 1.0
    oneh = oneh.astype(ml_dtypes.float8_e4m3)

    gidx_w = np.zeros((NB, 128, tot // 16), np.int16)
    for c in range(NB):
        gidx_w[c] = np.tile(_wrap16(gidx[c], tot), (8, 1))

    dinv_cols = np.zeros((NB, 128, NTL), np.float32)
    x_sh = np.zeros((NB, SH, x.shape[1]), np.float32)
    for c in range(NB):
        lo, hi = c * nsh, min((c + 1) * nsh, N)
        d = np.zeros(SH, np.float32)
        d[: hi - lo] = dinv[lo:hi]
        dinv_cols[c] = d.reshape(NTL, 128).T
        x_sh[c, : hi - lo] = x[lo:hi]

    return dict(
        N=N, nsh=nsh, SH=SH, NTL=NTL, NR=NR, QR=QR, tot=tot,
        nblocks=nblocks, NBM=NBM,
        P=tuple(map(tuple, P.tolist())),
        round_nblk=tuple(int(v) for v in round_nblk),
        sched=tuple(sched),
        gidx=gidx_w, oneh=oneh, dinv_cols=dinv_cols, x_sh=x_sh,
    )


# -------------------------------------------------------------- device side


def build_kernel(N, SH, NTL, NR, QR, tot, nblocks, NBM, P, round_nblk, sched,
                 D=128):
    nc = bacc.Bacc("TRN2", target_bir_lowering=False, debug=False,
                   num_devices=NB, num_swdge_queues=4)
    rg = [list(range(NB))]
    chunks512 = [(o, min(512, SH - o)) for o in range(0, SH, 512)]
    MTC = max(max(row) for row in P)               # max slots per call

    x_in = nc.dram_tensor("x", [SH, D], F32, kind="ExternalInput")
    gidx_in = nc.dram_tensor("gidx", [128, tot // 16], I16, kind="ExternalInput")
    oneh_in = nc.dram_tensor("oneh", [128, nblocks * 128], mybir.dt.float8e4,
                             kind="ExternalInput")
    dinv_in = nc.dram_tensor("dinv_cols", [128, NTL], F32, kind="ExternalInput")
    w_in = [nc.dram_tensor(f"w{l}", [D, D], F32, kind="ExternalInput")
            for l in range(2)]
    gam_in = [nc.dram_tensor(f"gamma{l}", [D, 1], F32, kind="ExternalInput")
              for l in range(2)]
    bet_in = [nc.dram_tensor(f"beta{l}", [D, 1], F32, kind="ExternalInput")
              for l in range(2)]
    a_in = [nc.dram_tensor(f"a{l}", [D, 1], F32, kind="ExternalInput")
            for l in range(2)]
    out_t = nc.dram_tensor("out", [SH, D], F32, kind="ExternalOutput")

    vloc = nc.dram_tensor("vloc", [SH, D], BF16)
    vfull = nc.dram_tensor("vfull", [NB * SH, D], BF16, addr_space="Shared")
    stats_in = nc.dram_tensor("stats_in", [D, 2], F32)
    stats_out = nc.dram_tensor("stats_out", [D, 2], F32, addr_space="Shared")

    out_r = out_t.ap().rearrange("(t p) f -> t p f", p=128)
    x_r = x_in.ap().rearrange("(t p) f -> t p f", p=128)
    vloc_r = vloc.ap().rearrange("(t p) f -> t p f", p=128)

    with tile.TileContext(nc) as tc:
        with (
            tc.tile_pool(name="pers", bufs=1) as PE_,
            tc.tile_pool(name="act", bufs=1) as PA,
            tc.tile_pool(name="msg", bufs=2) as PM,
            tc.tile_pool(name="work", bufs=3) as PW,
            tc.tile_pool(name="small", bufs=2) as PS,
            tc.tile_pool(name="psV", bufs=1, space="PSUM") as PV,
            tc.tile_pool(name="psA", bufs=4, space="PSUM") as PP,
            tc.tile_pool(name="psT", bufs=2, space="PSUM") as PT,
        ):
            ident = PE_.tile([128, 128], F32, tag="ident")
            make_identity(nc, ident[:])
            gidx_sb = PE_.tile([128, tot // 16], I16, tag="gidx")
            nc.sync.dma_start(gidx_sb[:], gidx_in.ap())

            dinv_sb = PE_.tile([128, NTL], F32, tag="dinv")
            nc.sync.dma_start(dinv_sb[:], dinv_in.ap())
            w_sb, gam_sb, bet_sb, a_sb = [], [], [], []
            for l in range(2):
                w_sb.append(PE_.tile([128, 128], F32, tag=f"w{l}", name=f"w{l}_sb"))
                nc.sync.dma_start(w_sb[l][:], w_in[l].ap())
                gam_sb.append(PE_.tile([128, 1], F32, tag=f"g{l}", name=f"g{l}_sb"))
                nc.sync.dma_start(gam_sb[l][:], gam_in[l].ap())
                bet_sb.append(PE_.tile([128, 1], F32, tag=f"b{l}", name=f"b{l}_sb"))
                nc.sync.dma_start(bet_sb[l][:], bet_in[l].ap())
                a_sb.append(PE_.tile([128, 1], F32, tag=f"a{l}", name=f"a{l}_sb"))
                nc.sync.dma_start(a_sb[l][:], a_in[l].ap())
            zero_sb = PE_.tile([128, 128], F32, tag="zero")
            nc.vector.memset(zero_sb[:], 0.0)
            eps_sb = PE_.tile([128, 1], F32, tag="eps")
            nc.vector.memset(eps_sb[:], EPS)

            actT = PA.tile([128, SH], F32, tag="actT")        # h [feat, rows]
            vself = PA.tile([128, NTL, 128], BF16, tag="vself")  # dinv^2*v rows

            # ---- load x, transpose into actT
            for t in range(NTL):
                xt = PW.tile([128, 128], F32, tag="xt")
                nc.sync.dma_start(xt[:], x_r[t])
                tp = PT.tile([128, 128], F32, tag="tp")
                nc.tensor.transpose(out=tp[:], in_=xt[:], identity=ident[:])
                nc.vector.tensor_copy(actT[:, 128 * t : 128 * (t + 1)], tp[:])

            for l in range(2):
                # ---- v' = dinv * (h @ W) -> bf16 vloc + bf16 self term
                for (o, cw) in chunks512:
                    vp = PV.tile([128, 512], F32, tag="vp")
                    nc.tensor.matmul(out=vp[:, :cw], lhsT=w_sb[l][:],
                                     rhs=actT[:, o : o + cw],
                                     start=True, stop=True)
                    vt = PW.tile([128, 512], F32, tag="vt")
                    nc.vector.tensor_copy(vt[:, :cw], vp[:, :cw])
                    for s in range(0, cw, 128):
                        t = (o + s) // 128
                        tp = PT.tile([128, 128], F32, tag="tp")
                        nc.tensor.transpose(out=tp[:], in_=vt[:, s : s + 128],
                                            identity=ident[:])
                        vv = PW.tile([128, 128], BF16, tag="vv")
                        nc.vector.tensor_scalar(
                            vv[:], tp[:], dinv_sb[:, t : t + 1], None,
                            op0=mybir.AluOpType.mult)
                        nc.sync.dma_start(vloc_r[t], vv[:])
                        nc.vector.tensor_scalar(
                            vself[:, t], vv[:], dinv_sb[:, t : t + 1], None,
                            op0=mybir.AluOpType.mult)

                # ---- halo exchange
                nc.gpsimd.collective_compute(
                    "AllGather", mybir.AluOpType.bypass, replica_groups=rg,
                    ins=[vloc.ap().opt()], outs=[vfull.ap().opt()])

                # ---- gather + one-hot matmul segment sum
                soff = 0          # stream slot offset
                blk0 = 0          # oneh block offset
                for r in range(NR):
                    tiles = list(range(r * RT, min((r + 1) * RT, NTL)))
                    nblk_r = round_nblk[r]
                    ohl = PM.tile([128, NBM, 128], mybir.dt.float8e4,
                                  tag="ohl")
                    if nblk_r:
                        nc.sync.dma_start(
                            ohl[:, :nblk_r, :],
                            oneh_in.ap()[:, blk0 * 128 : (blk0 + nblk_r) * 128])
                    mts = []
                    for q in range(NQ):
                        sz = P[r][q]
                        mt = PM.tile([128, MTC // 128, 128], BF16,
                                     tag=f"mt{q}")
                        nc.gpsimd.dma_gather(
                            out_ap=mt[:, : sz // 128, :],
                            in_ap=vfull.ap()[q * QR : (q + 1) * QR, :],
                            idxs_ap=gidx_sb[:, soff // 16 : (soff + sz) // 16],
                            num_idxs=sz, num_idxs_reg=sz, elem_size=D,
                            single_packet=False, queue_num=q)
                        mts.append(mt)
                        soff += sz
                    for ti, t in enumerate(tiles):
                        lst = sched[r][ti]
                        if not lst:
                            sc = PW.tile([128, 128], F32, tag="sc")
                            nc.vector.tensor_copy(sc[:], vself[:, t])
                        else:
                            acc = PP.tile([128, 128], F32, tag="acc")
                            nb = len(lst)
                            for i, (q, k, rb) in enumerate(lst):
                                nc.tensor.matmul(
                                    out=acc[:], lhsT=ohl[:, rb, :],
                                    rhs=mts[q][:, k, :],
                                    start=(i == 0),
                                    stop=(i == nb - 1))
                            sc = PW.tile([128, 128], F32, tag="sc")
                            nc.vector.scalar_tensor_tensor(
                                out=sc[:], in0=acc[:],
                                scalar=dinv_sb[:, t : t + 1],
                                in1=vself[:, t],
                                op0=mybir.AluOpType.mult,
                                op1=mybir.AluOpType.add)
                        tp = PT.tile([128, 128], F32, tag="tp")
                        nc.tensor.transpose(out=tp[:], in_=sc[:],
                                            identity=ident[:])
                        nc.scalar.activation(
                            out=actT[:, 128 * t : 128 * (t + 1)], in_=tp[:],
                            func=mybir.ActivationFunctionType.Copy)
                    blk0 += nblk_r

                # ---- BN stats (biased over real N rows; pad rows are 0)
                nk = len(chunks512)
                sumc = PS.tile([128, nk], F32, tag="sumc")
                sqc = PS.tile([128, nk], F32, tag="sqc")
                for k, (o, cw) in enumerate(chunks512):
                    nc.vector.tensor_reduce(
                        out=sumc[:, k : k + 1], in_=actT[:, o : o + cw],
                        axis=mybir.AxisListType.X, op=mybir.AluOpType.add)
                    sq = PW.tile([128, 512], F32, tag="sq")
                    nc.scalar.activation(
                        out=sq[:, :cw], in_=actT[:, o : o + cw],
                        func=mybir.ActivationFunctionType.Square,
                        bias=zero_sb[:, 0:1],
                        accum_out=sqc[:, k : k + 1])
                stats_sb = PS.tile([128, 2], F32, tag="stats")
                nc.vector.tensor_reduce(out=stats_sb[:, 0:1], in_=sumc[:],
                                        axis=mybir.AxisListType.X,
                                        op=mybir.AluOpType.add)
                nc.vector.tensor_reduce(out=stats_sb[:, 1:2], in_=sqc[:],
                                        axis=mybir.AxisListType.X,
                                        op=mybir.AluOpType.add)
                nc.sync.dma_start(stats_in.ap(), stats_sb[:])
                nc.gpsimd.collective_compute(
                    "AllReduce", mybir.AluOpType.add, replica_groups=rg,
                    ins=[stats_in.ap().opt()], outs=[stats_out.ap().opt()])
                stats2 = PS.tile([128, 2], F32, tag="stats2")
                nc.sync.dma_start(stats2[:], stats_out.ap())

                # ---- BN affine params
                mu = PS.tile([128, 1], F32, tag="mu")
                nc.vector.tensor_scalar(mu[:], stats2[:, 0:1], 1.0 / N, None,
                                        op0=mybir.AluOpType.mult)
                e2 = PS.tile([128, 1], F32, tag="e2")
                nc.vector.tensor_scalar(e2[:], stats2[:, 1:2], 1.0 / N, None,
                                        op0=mybir.AluOpType.mult)
                var = PS.tile([128, 1], F32, tag="var")
                nc.vector.scalar_tensor_tensor(
                    out=var[:], in0=mu[:], scalar=-1.0, in1=mu[:],
                    op0=mybir.AluOpType.mult, op1=mybir.AluOpType.mult)
                nc.vector.tensor_tensor(out=var[:], in0=e2[:], in1=var[:],
                                        op=mybir.AluOpType.add)
                sd = PS.tile([128, 1], F32, tag="sd")
                nc.scalar.activation(out=sd[:], in_=var[:],
                                     func=mybir.ActivationFunctionType.Sqrt,
                                     bias=eps_sb[:, 0:1])
                rinv = PS.tile([128, 1], F32, tag="rinv")
                nc.vector.reciprocal(rinv[:], sd[:])
                alpha = PS.tile([128, 1], F32, tag="alpha")
                nc.vector.tensor_tensor(out=alpha[:], in0=gam_sb[l][:],
                                        in1=rinv[:], op=mybir.AluOpType.mult)
                bias_p = PS.tile([128, 1], F32, tag="biasp")
                nc.vector.scalar_tensor_tensor(
                    out=bias_p[:], in0=alpha[:], scalar=-1.0, in1=mu[:],
                    op0=mybir.AluOpType.mult, op1=mybir.AluOpType.mult)
                nc.vector.tensor_tensor(out=bias_p[:], in0=bet_sb[l][:],
                                        in1=bias_p[:], op=mybir.AluOpType.add)
                nalpha = PS.tile([128, 1], F32, tag="nalpha")
                nc.vector.tensor_scalar(nalpha[:], alpha[:], -1.0, None,
                                        op0=mybir.AluOpType.mult)
                nbias = PS.tile([128, 1], F32, tag="nbias")
                nc.vector.tensor_scalar(nbias[:], bias_p[:], -1.0, None,
                                        op0=mybir.AluOpType.mult)
                na = PS.tile([128, 1], F32, tag="na")
                nc.vector.tensor_scalar(na[:], a_sb[l][:], -1.0, None,
                                        op0=mybir.AluOpType.mult)

                # ---- fused BN + PReLU: y = relu(z) - a*relu(-z)
                for (o, cw) in chunks512:
                    pos = PW.tile([128, 512], F32, tag="pos")
                    nc.scalar.activation(
                        out=pos[:, :cw], in_=actT[:, o : o + cw],
                        func=mybir.ActivationFunctionType.Relu,
                        bias=bias_p[:, :1], scale=alpha[:, :1])
                    neg = PW.tile([128, 512], F32, tag="neg")
                    nc.scalar.activation(
                        out=neg[:, :cw], in_=actT[:, o : o + cw],
                        func=mybir.ActivationFunctionType.Relu,
                        bias=nbias[:, :1], scale=nalpha[:, :1])
                    nc.vector.scalar_tensor_tensor(
                        out=actT[:, o : o + cw], in0=neg[:, :cw],
                        scalar=na[:, :1], in1=pos[:, :cw],
                        op0=mybir.AluOpType.mult, op1=mybir.AluOpType.add)

            # ---- write h2 back as [rows, feat]
            for t in range(NTL):
                tp = PT.tile([128, 128], F32, tag="tp")
                nc.tensor.transpose(out=tp[:],
                                    in_=actT[:, 128 * t : 128 * (t + 1)],
                                    identity=ident[:])
                ot = PW.tile([128, 128], F32, tag="ot")
                nc.vector.tensor_copy(ot[:], tp[:])
                nc.sync.dma_start(out_r[t], ot[:])

    nc.compile()
    return nc


# ------------------------------------------------------------------- driver

_CACHE: dict = {}


def _get_compiled(pre):
    key = (pre["N"], pre["SH"], pre["tot"], pre["nblocks"], pre["P"],
           pre["sched"])
    if key not in _CACHE:
        import os
        nc = build_kernel(pre["N"], pre["SH"], pre["NTL"], pre["NR"],
                          pre["QR"], pre["tot"], pre["nblocks"], pre["NBM"],
                          pre["P"], pre["round_nblk"], pre["sched"])
        if not os.environ.get("KN_SIM"):
            nc.m = get_hw_module(nc.m)
        _CACHE[key] = nc
    return _CACHE[key]


def make_in_maps(pre, w0, gamma0, beta0, a0, w1, gamma1, beta1, a1):
    def col(v):
        return np.ascontiguousarray(np.asarray(v, np.float32).reshape(-1, 1))

    def rep(v):
        return np.full((128, 1), np.float32(np.asarray(v).reshape(-1)[0]),
                       np.float32)

    maps = []
    for c in range(NB):
        maps.append({
            "x": pre["x_sh"][c],
            "gidx": pre["gidx"][c],
            "oneh": pre["oneh"][c],
            "dinv_cols": pre["dinv_cols"][c],
            "w0": np.ascontiguousarray(np.asarray(w0, np.float32)),
            "w1": np.ascontiguousarray(np.asarray(w1, np.float32)),
            "gamma0": col(gamma0), "beta0": col(beta0), "a0": rep(a0),
            "gamma1": col(gamma1), "beta1": col(beta1), "a1": rep(a1),
        })
    return maps


def kernel(x, edge_index, w0, b0, gamma0, beta0, a0,
           w1, b1, gamma1, beta1, a1, _trace=False):
    x = np.asarray(x, np.float32)
    edge_index = np.asarray(edge_index, np.int64)
    pre = preprocess(x, edge_index)
    nc = _get_compiled(pre)
    in_maps = make_in_maps(pre, w0, gamma0, beta0, a0, w1, gamma1, beta1, a1)
    res = bass_utils.run_bass_kernel_spmd(
        nc, in_maps, core_ids=list(range(NB)), trace=_trace)
    nsh, N = pre["nsh"], pre["N"]
    out = np.concatenate([res.results[c]["out"][:nsh] for c in range(NB)],
                         axis=0)[:N]
    if _trace:
        kernel.last_results = res
    return np.ascontiguousarray(out)


# revision 10
# speedup vs baseline: 3.1720x; 1.1906x over previous
"""GCN encoder (2x GCNConv + BatchNorm + PReLU) on 8 Trainium2 NeuronCores.

v3: scatter-free design, unpadded tile-impure gather chunks, 4 SWDGE queues.
  - nodes sharded contiguously across 8 cores (12500 real + pad -> 12544);
  - per layer: v' = dinv * (h @ W) computed locally, AllGather of the bf16
    v' table (halo exchange);
  - edges dst-sharded (owner of dst), stream sorted by (round, src quarter,
    dst tile); per (round, quarter) ONE gather call (queue_num=q) with only
    trailing padding -> ~107k gather descriptors/layer vs 150k in v2;
  - chunks of 128 edge slots may span tiles: per (chunk, tile) a host-built
    fp8 one-hot block S[slot, j] = (edge slot -> tile row j), streamed per
    round, used as PSUM-accumulated matmuls acc_t[j, d] += S^T @ msg;
  - conv = dinv[dst] * acc + dinv^2 * v (self-loop analytic); BN stats via
    free-dim reduce + AllReduce; fused BN+PReLU via two ACT Relu passes and
    one DVE scalar_tensor_tensor.

norm_e = dinv[src]*dinv[dst] is separable: dinv[src] folded into the
gathered table, dinv[dst] at read-out.  BatchNorm cancels the conv bias,
so b0/b1 are accepted and ignored.
"""

import numpy as np

import concourse.bass as bass
import concourse.bacc as bacc
import concourse.tile as tile
from concourse import mybir
from concourse import bass_utils
from concourse.masks import make_identity
from concourse.bass_interp import get_hw_module

F32 = mybir.dt.float32
BF16 = mybir.dt.bfloat16
I16 = mybir.dt.int16
EPS = 1e-5
NB = 8            # cores
NQ = 4            # src quarters (int16 idx limit)
RT = 7            # dst tiles per round (PSUM budget)


def _wrap16(vals: np.ndarray, cap: int) -> np.ndarray:
    assert vals.shape[0] == cap and cap % 16 == 0
    return np.ascontiguousarray(vals.reshape(cap // 16, 16).T)


def preprocess(x: np.ndarray, edge_index: np.ndarray):
    """Sort edges by (core, round, quarter, tile); pad per (round, quarter)
    call only at the tail; build per-(chunk, tile) one-hot blocks with
    cross-core-max chunk spans and the matmul schedule."""
    N = x.shape[0]
    nsh = (N + NB - 1) // NB                   # 12500 real rows/shard
    SH = ((nsh + 127) // 128) * 128            # 12544 padded
    NTL = SH // 128                            # 98 local tiles
    NR = (NTL + RT - 1) // RT                  # rounds
    QR = (NB * SH) // NQ                       # rows per src quarter

    src = edge_index[0].astype(np.int64)
    dst = edge_index[1].astype(np.int64)
    deg = np.bincount(dst, minlength=N) + 1.0  # +1 self-loop
    dinv = (1.0 / np.sqrt(deg)).astype(np.float32)

    gsrc = (src // nsh) * SH + (src % nsh)     # padded-global row id
    c_of = dst // nsh
    dloc = dst % nsh
    q_of = gsrc // QR
    t_of = dloc // 128
    j_of = dloc % 128
    r_of = t_of // RT

    # edge stream order: (core, round, quarter, tile)
    key = ((c_of * NR + r_of) * NQ + q_of) * NTL + t_of
    order = np.argsort(key, kind="stable")
    ks = key[order]
    gq = (gsrc % QR)[order].astype(np.int16)
    jj = j_of[order].astype(np.int64)

    # per (c, r, q) counts -> shared static call sizes (cross-core max)
    crq = ks // NTL                            # (c*NR + r)*NQ + q
    cnt_crq = np.bincount(crq, minlength=NB * NR * NQ).reshape(NB, NR, NQ)
    P = (np.ceil(cnt_crq.max(axis=0) / 128).astype(np.int64)) * 128  # [NR,NQ]
    P = np.maximum(P, 128)
    call_off = np.zeros(NR * NQ + 1, np.int64)
    np.cumsum(P.reshape(-1), out=call_off[1:])
    tot = int(call_off[-1])                    # padded slots per core / layer

    # slot of each edge inside its (c,r,q) call
    crq_start = np.zeros(NB * NR * NQ + 1, np.int64)
    np.cumsum(cnt_crq.reshape(-1), out=crq_start[1:])
    slot = np.arange(len(ks)) - crq_start[crq]
    chunk = slot // 128

    # gather idx stream (pad idx 0)
    gidx = np.zeros((NB, tot), np.int16)
    c_e = crq // (NR * NQ)
    rq_e = crq % (NR * NQ)
    gidx[c_e, call_off[rq_e] + slot] = gq

    # per-(c,r,q,t) slot ranges -> global chunk spans per (r,q,t)
    uk, ui, uc = np.unique(ks, return_index=True, return_counts=True)
    a_c = slot[ui]
    b_c = a_c + uc
    rqt_u = (uk % (NR * NQ)) * NTL + (uk % NTL)
    # careful: rqt index = ((r*NQ)+q)*NTL + t; uk = ((c*NR+r)*NQ+q)*NTL+t
    rq_u = (uk // NTL) % (NR * NQ)
    t_u = uk % NTL
    rqt_u = rq_u * NTL + t_u
    NG = NR * NQ * NTL
    lo_g = np.full(NG, 2**31, np.int64)
    hi_g = np.zeros(NG, np.int64)
    np.minimum.at(lo_g, rqt_u, a_c // 128)
    np.maximum.at(hi_g, rqt_u, (b_c + 127) // 128)

    # block list ordered (r, t, q, k); schedule per (r, t)
    rb_start = np.zeros(NG, np.int64)          # block index within round
    round_nblk = np.zeros(NR, np.int64)
    round_blk0 = np.zeros(NR + 1, np.int64)
    sched = []                                 # sched[r][t-rel] = [(q,k,rb)..]
    for r in range(NR):
        tiles = range(r * RT, min((r + 1) * RT, NTL))
        rb = 0
        srt = []
        for t in tiles:
            lst = []
            for q in range(NQ):
                g = (r * NQ + q) * NTL + t
                if hi_g[g] > lo_g[g]:
                    rb_start[g] = rb
                    for k in range(int(lo_g[g]), int(hi_g[g])):
                        lst.append((q, k, rb))
                        rb += 1
            srt.append(tuple(lst))
        sched.append(tuple(srt))
        round_nblk[r] = rb
        round_blk0[r + 1] = round_blk0[r] + rb
    nblocks = int(round_blk0[-1])
    NBM = int(round_nblk.max())

    # one-hot blocks, fp8: oneh[c, slot%128, blk*128 + j]
    import ml_dtypes
    g_e = rq_e * NTL + t_of[order]
    blk_e = round_blk0[rq_e // NQ] + rb_start[g_e] + (chunk - lo_g[g_e])
    oneh = np.zeros((NB, 128, nblocks * 128), np.float32)
    oneh[c_e, slot % 128, blk_e * 128 + jj] = 1.0
    oneh = oneh.astype(ml_dtypes.float8_e4m3)

    gidx_w = np.zeros((NB, 128, tot // 16), np.int16)
    for c in range(NB):
        gidx_w[c] = np.tile(_wrap16(gidx[c], tot), (8, 1))

    dinv_cols = np.zeros((NB, 128, NTL), np.float32)
    x_sh = np.zeros((NB, 128, SH), np.float32)      # host-transposed [feat, rows]
    for c in range(NB):
        lo, hi = c * nsh, min((c + 1) * nsh, N)
        d = np.zeros(SH, np.float32)
        d[: hi - lo] = dinv[lo:hi]
        dinv_cols[c] = d.reshape(NTL, 128).T
        x_sh[c, :, : hi - lo] = x[lo:hi].T

    return dict(
        N=N, nsh=nsh, SH=SH, NTL=NTL, NR=NR, QR=QR, tot=tot,
        nblocks=nblocks, NBM=NBM,
        P=tuple(map(tuple, P.tolist())),
        round_nblk=tuple(int(v) for v in round_nblk),
        sched=tuple(sched),
        gidx=gidx_w, oneh=oneh, dinv_cols=dinv_cols, x_sh=x_sh,
    )


# -------------------------------------------------------------- device side


def build_kernel(N, SH, NTL, NR, QR, tot, nblocks, NBM, P, round_nblk, sched,
                 D=128):
    nc = bacc.Bacc("TRN2", target_bir_lowering=False, debug=False,
                   num_devices=NB, num_swdge_queues=4)
    rg = [list(range(NB))]
    chunks512 = [(o, min(512, SH - o)) for o in range(0, SH, 512)]
    MTC = max(max(row) for row in P)               # max slots per call

    x_in = nc.dram_tensor("x", [D, SH], F32, kind="ExternalInput")
    gidx_in = nc.dram_tensor("gidx", [128, tot // 16], I16, kind="ExternalInput")
    oneh_in = nc.dram_tensor("oneh", [128, nblocks * 128], mybir.dt.float8e4,
                             kind="ExternalInput")
    dinv_in = nc.dram_tensor("dinv_cols", [128, NTL], F32, kind="ExternalInput")
    w_in = [nc.dram_tensor(f"w{l}", [D, D], F32, kind="ExternalInput")
            for l in range(2)]
    gam_in = [nc.dram_tensor(f"gamma{l}", [D, 1], F32, kind="ExternalInput")
              for l in range(2)]
    bet_in = [nc.dram_tensor(f"beta{l}", [D, 1], F32, kind="ExternalInput")
              for l in range(2)]
    a_in = [nc.dram_tensor(f"a{l}", [D, 1], F32, kind="ExternalInput")
            for l in range(2)]
    out_t = nc.dram_tensor("out", [SH, D], F32, kind="ExternalOutput")

    vloc = nc.dram_tensor("vloc", [SH, D], BF16)
    vfull = nc.dram_tensor("vfull", [NB * SH, D], BF16, addr_space="Shared")
    stats_in = nc.dram_tensor("stats_in", [D, 2], F32)
    stats_out = nc.dram_tensor("stats_out", [D, 2], F32, addr_space="Shared")

    out_r = out_t.ap().rearrange("(t p) f -> p t f", p=128)   # batched writes
    vloc_r = vloc.ap().rearrange("(t p) f -> p t f", p=128)

    with tile.TileContext(nc) as tc:
        with (
            tc.tile_pool(name="pers", bufs=1) as PE_,
            tc.tile_pool(name="act", bufs=1) as PA,
            tc.tile_pool(name="msg", bufs=3) as PM,
            tc.tile_pool(name="onehp", bufs=2) as PO,
            tc.tile_pool(name="work", bufs=3) as PW,
            tc.tile_pool(name="small", bufs=2) as PS,
            tc.tile_pool(name="psV", bufs=1, space="PSUM") as PV,
            tc.tile_pool(name="psA", bufs=4, space="PSUM") as PP,
            tc.tile_pool(name="psT", bufs=2, space="PSUM") as PT,
        ):
            ident = PE_.tile([128, 128], F32, tag="ident")
            make_identity(nc, ident[:])
            gidx_sb = PE_.tile([128, tot // 16], I16, tag="gidx")
            nc.sync.dma_start(gidx_sb[:], gidx_in.ap())

            dinv_sb = PE_.tile([128, NTL], F32, tag="dinv")
            nc.sync.dma_start(dinv_sb[:], dinv_in.ap())
            w_sb, gam_sb, bet_sb, a_sb = [], [], [], []
            for l in range(2):
                w_sb.append(PE_.tile([128, 128], F32, tag=f"w{l}", name=f"w{l}_sb"))
                nc.sync.dma_start(w_sb[l][:], w_in[l].ap())
                gam_sb.append(PE_.tile([128, 1], F32, tag=f"g{l}", name=f"g{l}_sb"))
                nc.sync.dma_start(gam_sb[l][:], gam_in[l].ap())
                bet_sb.append(PE_.tile([128, 1], F32, tag=f"b{l}", name=f"b{l}_sb"))
                nc.sync.dma_start(bet_sb[l][:], bet_in[l].ap())
                a_sb.append(PE_.tile([128, 1], F32, tag=f"a{l}", name=f"a{l}_sb"))
                nc.sync.dma_start(a_sb[l][:], a_in[l].ap())
            zero_sb = PE_.tile([128, 128], F32, tag="zero")
            nc.vector.memset(zero_sb[:], 0.0)
            eps_sb = PE_.tile([128, 1], F32, tag="eps")
            nc.vector.memset(eps_sb[:], EPS)

            actT = PA.tile([128, SH], F32, tag="actT")        # h [feat, rows]
            vself = PA.tile([128, NTL, 128], BF16, tag="vself")  # dinv^2*v rows

            # ---- load x (host-transposed) straight into actT
            for (o, cw) in chunks512:
                nc.sync.dma_start(actT[:, o : o + cw], x_in.ap()[:, o : o + cw])

            for l in range(2):
                # ---- v' = dinv * (h @ W) -> bf16 vloc + bf16 self term
                for (o, cw) in chunks512:
                    vp = PV.tile([128, 512], F32, tag="vp")
                    nc.tensor.matmul(out=vp[:, :cw], lhsT=w_sb[l][:],
                                     rhs=actT[:, o : o + cw],
                                     start=True, stop=True)
                    vt = PW.tile([128, 512], F32, tag="vt")
                    nc.vector.tensor_copy(vt[:, :cw], vp[:, :cw])
                    vvb = PW.tile([128, 4, 128], BF16, tag="vvb")
                    for s in range(0, cw, 128):
                        t = (o + s) // 128
                        tp = PT.tile([128, 128], F32, tag="tp")
                        nc.tensor.transpose(out=tp[:], in_=vt[:, s : s + 128],
                                            identity=ident[:])
                        nc.vector.tensor_scalar(
                            vvb[:, s // 128], tp[:], dinv_sb[:, t : t + 1],
                            None, op0=mybir.AluOpType.mult)
                        nc.vector.tensor_scalar(
                            vself[:, t], vvb[:, s // 128],
                            dinv_sb[:, t : t + 1], None,
                            op0=mybir.AluOpType.mult)
                    nc.sync.dma_start(
                        vloc_r[:, o // 128 : o // 128 + cw // 128, :],
                        vvb[:, : cw // 128, :])

                # ---- halo exchange
                nc.gpsimd.collective_compute(
                    "AllGather", mybir.AluOpType.bypass, replica_groups=rg,
                    ins=[vloc.ap().opt()], outs=[vfull.ap().opt()])

                # ---- gather + one-hot matmul segment sum
                soff = 0          # stream slot offset
                blk0 = 0          # oneh block offset
                for r in range(NR):
                    tiles = list(range(r * RT, min((r + 1) * RT, NTL)))
                    nblk_r = round_nblk[r]
                    ohl = PO.tile([128, NBM, 128], mybir.dt.float8e4,
                                  tag="ohl")
                    if nblk_r:
                        nc.sync.dma_start(
                            ohl[:, :nblk_r, :],
                            oneh_in.ap()[:, blk0 * 128 : (blk0 + nblk_r) * 128])
                    mts = []
                    for q in range(NQ):
                        sz = P[r][q]
                        mt = PM.tile([128, MTC // 128, 128], BF16,
                                     tag=f"mt{q}")
                        nc.gpsimd.dma_gather(
                            out_ap=mt[:, : sz // 128, :],
                            in_ap=vfull.ap()[q * QR : (q + 1) * QR, :],
                            idxs_ap=gidx_sb[:, soff // 16 : (soff + sz) // 16],
                            num_idxs=sz, num_idxs_reg=sz, elem_size=D,
                            single_packet=False, queue_num=q)
                        mts.append(mt)
                        soff += sz
                    for ti, t in enumerate(tiles):
                        lst = sched[r][ti]
                        if not lst:
                            sc = PW.tile([128, 128], F32, tag="sc")
                            nc.vector.tensor_copy(sc[:], vself[:, t])
                        else:
                            acc = PP.tile([128, 128], F32, tag="acc")
                            nb = len(lst)
                            for i, (q, k, rb) in enumerate(lst):
                                nc.tensor.matmul(
                                    out=acc[:], lhsT=ohl[:, rb, :],
                                    rhs=mts[q][:, k, :],
                                    start=(i == 0),
                                    stop=(i == nb - 1))
                            sc = PW.tile([128, 128], F32, tag="sc")
                            nc.vector.scalar_tensor_tensor(
                                out=sc[:], in0=acc[:],
                                scalar=dinv_sb[:, t : t + 1],
                                in1=vself[:, t],
                                op0=mybir.AluOpType.mult,
                                op1=mybir.AluOpType.add)
                        tp = PT.tile([128, 128], F32, tag="tp")
                        nc.tensor.transpose(out=tp[:], in_=sc[:],
                                            identity=ident[:])
                        nc.scalar.activation(
                            out=actT[:, 128 * t : 128 * (t + 1)], in_=tp[:],
                            func=mybir.ActivationFunctionType.Copy)
                    blk0 += nblk_r

                # ---- BN stats (biased over real N rows; pad rows are 0)
                nk = len(chunks512)
                sumc = PS.tile([128, nk], F32, tag="sumc")
                sqc = PS.tile([128, nk], F32, tag="sqc")
                for k, (o, cw) in enumerate(chunks512):
                    nc.vector.tensor_reduce(
                        out=sumc[:, k : k + 1], in_=actT[:, o : o + cw],
                        axis=mybir.AxisListType.X, op=mybir.AluOpType.add)
                    sq = PW.tile([128, 512], F32, tag="sq")
                    nc.scalar.activation(
                        out=sq[:, :cw], in_=actT[:, o : o + cw],
                        func=mybir.ActivationFunctionType.Square,
                        bias=zero_sb[:, 0:1],
                        accum_out=sqc[:, k : k + 1])
                stats_sb = PS.tile([128, 2], F32, tag="stats")
                nc.vector.tensor_reduce(out=stats_sb[:, 0:1], in_=sumc[:],
                                        axis=mybir.AxisListType.X,
                                        op=mybir.AluOpType.add)
                nc.vector.tensor_reduce(out=stats_sb[:, 1:2], in_=sqc[:],
                                        axis=mybir.AxisListType.X,
                                        op=mybir.AluOpType.add)
                nc.sync.dma_start(stats_in.ap(), stats_sb[:])
                nc.gpsimd.collective_compute(
                    "AllReduce", mybir.AluOpType.add, replica_groups=rg,
                    ins=[stats_in.ap().opt()], outs=[stats_out.ap().opt()])
                stats2 = PS.tile([128, 2], F32, tag="stats2")
                nc.sync.dma_start(stats2[:], stats_out.ap())

                # ---- BN affine params
                mu = PS.tile([128, 1], F32, tag="mu")
                nc.vector.tensor_scalar(mu[:], stats2[:, 0:1], 1.0 / N, None,
                                        op0=mybir.AluOpType.mult)
                e2 = PS.tile([128, 1], F32, tag="e2")
                nc.vector.tensor_scalar(e2[:], stats2[:, 1:2], 1.0 / N, None,
                                        op0=mybir.AluOpType.mult)
                var = PS.tile([128, 1], F32, tag="var")
                nc.vector.scalar_tensor_tensor(
                    out=var[:], in0=mu[:], scalar=-1.0, in1=mu[:],
                    op0=mybir.AluOpType.mult, op1=mybir.AluOpType.mult)
                nc.vector.tensor_tensor(out=var[:], in0=e2[:], in1=var[:],
                                        op=mybir.AluOpType.add)
                sd = PS.tile([128, 1], F32, tag="sd")
                nc.scalar.activation(out=sd[:], in_=var[:],
                                     func=mybir.ActivationFunctionType.Sqrt,
                                     bias=eps_sb[:, 0:1])
                rinv = PS.tile([128, 1], F32, tag="rinv")
                nc.vector.reciprocal(rinv[:], sd[:])
                alpha = PS.tile([128, 1], F32, tag="alpha")
                nc.vector.tensor_tensor(out=alpha[:], in0=gam_sb[l][:],
                                        in1=rinv[:], op=mybir.AluOpType.mult)
                bias_p = PS.tile([128, 1], F32, tag="biasp")
                nc.vector.scalar_tensor_tensor(
                    out=bias_p[:], in0=alpha[:], scalar=-1.0, in1=mu[:],
                    op0=mybir.AluOpType.mult, op1=mybir.AluOpType.mult)
                nc.vector.tensor_tensor(out=bias_p[:], in0=bet_sb[l][:],
                                        in1=bias_p[:], op=mybir.AluOpType.add)
                nalpha = PS.tile([128, 1], F32, tag="nalpha")
                nc.vector.tensor_scalar(nalpha[:], alpha[:], -1.0, None,
                                        op0=mybir.AluOpType.mult)
                nbias = PS.tile([128, 1], F32, tag="nbias")
                nc.vector.tensor_scalar(nbias[:], bias_p[:], -1.0, None,
                                        op0=mybir.AluOpType.mult)
                na = PS.tile([128, 1], F32, tag="na")
                nc.vector.tensor_scalar(na[:], a_sb[l][:], -1.0, None,
                                        op0=mybir.AluOpType.mult)

                # ---- fused BN + PReLU: y = relu(z) - a*relu(-z)
                for (o, cw) in chunks512:
                    pos = PW.tile([128, 512], F32, tag="pos")
                    nc.scalar.activation(
                        out=pos[:, :cw], in_=actT[:, o : o + cw],
                        func=mybir.ActivationFunctionType.Relu,
                        bias=bias_p[:, :1], scale=alpha[:, :1])
                    neg = PW.tile([128, 512], F32, tag="neg")
                    nc.scalar.activation(
                        out=neg[:, :cw], in_=actT[:, o : o + cw],
                        func=mybir.ActivationFunctionType.Relu,
                        bias=nbias[:, :1], scale=nalpha[:, :1])
                    nc.vector.scalar_tensor_tensor(
                        out=actT[:, o : o + cw], in0=neg[:, :cw],
                        scalar=na[:, :1], in1=pos[:, :cw],
                        op0=mybir.AluOpType.mult, op1=mybir.AluOpType.add)

            # ---- write h2 back as [rows, feat], batched 4 tiles per DMA
            for (o, cw) in chunks512:
                otb = PW.tile([128, 4, 128], F32, tag="otb")
                for s in range(0, cw, 128):
                    t = (o + s) // 128
                    tp = PT.tile([128, 128], F32, tag="tp")
                    nc.tensor.transpose(out=tp[:],
                                        in_=actT[:, 128 * t : 128 * (t + 1)],
                                        identity=ident[:])
                    nc.vector.tensor_copy(otb[:, s // 128], tp[:])
                nc.sync.dma_start(out_r[:, o // 128 : o // 128 + cw // 128, :],
                                  otb[:, : cw // 128, :])

    nc.compile()
    return nc


# ------------------------------------------------------------------- driver

_CACHE: dict = {}


def _get_compiled(pre):
    key = (pre["N"], pre["SH"], pre["tot"], pre["nblocks"], pre["P"],
           pre["sched"])
    if key not in _CACHE:
        import os
        nc = build_kernel(pre["N"], pre["SH"], pre["NTL"], pre["NR"],
                          pre["QR"], pre["tot"], pre["nblocks"], pre["NBM"],
                          pre["P"], pre["round_nblk"], pre["sched"])
        if not os.environ.get("KN_SIM"):
            nc.m = get_hw_module(nc.m)
        _CACHE[key] = nc
    return _CACHE[key]


def make_in_maps(pre, w0, gamma0, beta0, a0, w1, gamma1, beta1, a1):
    def col(v):
        return np.ascontiguousarray(np.asarray(v, np.float32).reshape(-1, 1))

    def rep(v):
        return np.full((128, 1), np.float32(np.asarray(v).reshape(-1)[0]),
                       np.float32)

    maps = []
    for c in range(NB):
        maps.append({
            "x": pre["x_sh"][c],
            "gidx": pre["gidx"][c],
            "oneh": pre["oneh"][c],
            "dinv_cols": pre["dinv_cols"][c],
            "w0": np.ascontiguousarray(np.asarray(w0, np.float32)),
            "w1": np.ascontiguousarray(np.asarray(w1, np.float32)),
            "gamma0": col(gamma0), "beta0": col(beta0), "a0": rep(a0),
            "gamma1": col(gamma1), "beta1": col(beta1), "a1": rep(a1),
        })
    return maps


def kernel(x, edge_index, w0, b0, gamma0, beta0, a0,
           w1, b1, gamma1, beta1, a1, _trace=False):
    x = np.asarray(x, np.float32)
    edge_index = np.asarray(edge_index, np.int64)
    pre = preprocess(x, edge_index)
    nc = _get_compiled(pre)
    in_maps = make_in_maps(pre, w0, gamma0, beta0, a0, w1, gamma1, beta1, a1)
    res = bass_utils.run_bass_kernel_spmd(
        nc, in_maps, core_ids=list(range(NB)), trace=_trace)
    nsh, N = pre["nsh"], pre["N"]
    out = np.concatenate([res.results[c]["out"][:nsh] for c in range(NB)],
                         axis=0)[:N]
    if _trace:
        kernel.last_results = res
    return np.ascontiguousarray(out)


# revision 16
# speedup vs baseline: 3.2434x; 1.0225x over previous
"""GCN encoder (2x GCNConv + BatchNorm + PReLU) on 8 Trainium2 NeuronCores.

v3: scatter-free design, unpadded tile-impure gather chunks, 4 SWDGE queues.
  - nodes sharded contiguously across 8 cores (12500 real + pad -> 12544);
  - per layer: v' = dinv * (h @ W) computed locally, AllGather of the bf16
    v' table (halo exchange);
  - edges dst-sharded (owner of dst), stream sorted by (round, src quarter,
    dst tile); per (round, quarter) ONE gather call (queue_num=q) with only
    trailing padding -> ~107k gather descriptors/layer vs 150k in v2;
  - chunks of 128 edge slots may span tiles: per (chunk, tile) a host-built
    fp8 one-hot block S[slot, j] = (edge slot -> tile row j), streamed per
    round, used as PSUM-accumulated matmuls acc_t[j, d] += S^T @ msg;
  - conv = dinv[dst] * acc + dinv^2 * v (self-loop analytic); BN stats via
    free-dim reduce + AllReduce; fused BN+PReLU via two ACT Relu passes and
    one DVE scalar_tensor_tensor.

norm_e = dinv[src]*dinv[dst] is separable: dinv[src] folded into the
gathered table, dinv[dst] at read-out.  BatchNorm cancels the conv bias,
so b0/b1 are accepted and ignored.
"""

import numpy as np

import concourse.bass as bass
import concourse.bacc as bacc
import concourse.tile as tile
from concourse import mybir
from concourse import bass_utils
from concourse.masks import make_identity
from concourse.bass_interp import get_hw_module

F32 = mybir.dt.float32
BF16 = mybir.dt.bfloat16
I16 = mybir.dt.int16
EPS = 1e-5
NB = 8            # cores
NQ = 4            # src quarters (int16 idx limit)
RT = 7            # dst tiles per round (PSUM budget)


def _wrap16(vals: np.ndarray, cap: int) -> np.ndarray:
    assert vals.shape[0] == cap and cap % 16 == 0
    return np.ascontiguousarray(vals.reshape(cap // 16, 16).T)


def preprocess(x: np.ndarray, edge_index: np.ndarray):
    """Sort edges by (core, round, quarter, tile); pad per (round, quarter)
    call only at the tail; build per-(chunk, tile) one-hot blocks with
    cross-core-max chunk spans and the matmul schedule."""
    N = x.shape[0]
    nsh = (N + NB - 1) // NB                   # 12500 real rows/shard
    SH = ((nsh + 127) // 128) * 128            # 12544 padded
    NTL = SH // 128                            # 98 local tiles
    NR = (NTL + RT - 1) // RT                  # rounds
    QR = (NB * SH) // NQ                       # rows per src quarter

    src = edge_index[0].astype(np.int64)
    dst = edge_index[1].astype(np.int64)
    deg = np.bincount(dst, minlength=N) + 1.0  # +1 self-loop
    dinv = (1.0 / np.sqrt(deg)).astype(np.float32)

    gsrc = (src // nsh) * SH + (src % nsh)     # padded-global row id
    c_of = dst // nsh
    dloc = dst % nsh
    q_of = gsrc // QR
    t_of = dloc // 128
    j_of = dloc % 128
    r_of = t_of // RT

    # edge stream order: (core, round, quarter, tile)
    key = ((c_of * NR + r_of) * NQ + q_of) * NTL + t_of
    order = np.argsort(key, kind="stable")
    ks = key[order]
    gq = (gsrc % QR)[order].astype(np.int16)
    jj = j_of[order].astype(np.int64)

    # per (c, r, q) counts -> shared static call sizes (cross-core max)
    crq = ks // NTL                            # (c*NR + r)*NQ + q
    cnt_crq = np.bincount(crq, minlength=NB * NR * NQ).reshape(NB, NR, NQ)
    P = (np.ceil(cnt_crq.max(axis=0) / 128).astype(np.int64)) * 128  # [NR,NQ]
    P = np.maximum(P, 128)
    call_off = np.zeros(NR * NQ + 1, np.int64)
    np.cumsum(P.reshape(-1), out=call_off[1:])
    tot = int(call_off[-1])                    # padded slots per core / layer

    # slot of each edge inside its (c,r,q) call
    crq_start = np.zeros(NB * NR * NQ + 1, np.int64)
    np.cumsum(cnt_crq.reshape(-1), out=crq_start[1:])
    slot = np.arange(len(ks)) - crq_start[crq]
    chunk = slot // 128

    # gather idx stream (pads use idx 0: cheap, keeps msg tiles initialized)
    gidx = np.zeros((NB, tot), np.int16)
    c_e = crq // (NR * NQ)
    rq_e = crq % (NR * NQ)
    gidx[c_e, call_off[rq_e] + slot] = gq

    # per-(c,r,q,t) slot ranges -> global chunk spans per (r,q,t)
    uk, ui, uc = np.unique(ks, return_index=True, return_counts=True)
    a_c = slot[ui]
    b_c = a_c + uc
    rqt_u = (uk % (NR * NQ)) * NTL + (uk % NTL)
    # careful: rqt index = ((r*NQ)+q)*NTL + t; uk = ((c*NR+r)*NQ+q)*NTL+t
    rq_u = (uk // NTL) % (NR * NQ)
    t_u = uk % NTL
    rqt_u = rq_u * NTL + t_u
    NG = NR * NQ * NTL
    lo_g = np.full(NG, 2**31, np.int64)
    hi_g = np.zeros(NG, np.int64)
    np.minimum.at(lo_g, rqt_u, a_c // 128)
    np.maximum.at(hi_g, rqt_u, (b_c + 127) // 128)

    # block list ordered (r, t, q, k); schedule per (r, t)
    rb_start = np.zeros(NG, np.int64)          # block index within round
    round_nblk = np.zeros(NR, np.int64)
    round_blk0 = np.zeros(NR + 1, np.int64)
    sched = []                                 # sched[r][t-rel] = [(q,k,rb)..]
    for r in range(NR):
        tiles = range(r * RT, min((r + 1) * RT, NTL))
        rb = 0
        srt = []
        for t in tiles:
            lst = []
            for q in range(NQ):
                g = (r * NQ + q) * NTL + t
                if hi_g[g] > lo_g[g]:
                    rb_start[g] = rb
                    for k in range(int(lo_g[g]), int(hi_g[g])):
                        lst.append((q, k, rb))
                        rb += 1
            srt.append(tuple(lst))
        sched.append(tuple(srt))
        round_nblk[r] = rb
        round_blk0[r + 1] = round_blk0[r] + rb
    nblocks = int(round_blk0[-1])
    NBM = int(round_nblk.max())

    # one-hot blocks, fp8: oneh[c, slot%128, blk*128 + j]
    import ml_dtypes
    g_e = rq_e * NTL + t_of[order]
    blk_e = round_blk0[rq_e // NQ] + rb_start[g_e] + (chunk - lo_g[g_e])
    oneh = np.zeros((NB, 128, nblocks * 128), np.float32)
    oneh[c_e, slot % 128, blk_e * 128 + jj] = 1.0
    oneh = oneh.astype(ml_dtypes.float8_e4m3)

    gidx_w = np.zeros((NB, 128, tot // 16), np.int16)
    for c in range(NB):
        gidx_w[c] = np.tile(_wrap16(gidx[c], tot), (8, 1))

    dinv_cols = np.zeros((NB, 128, NTL), np.float32)
    x_sh = np.zeros((NB, 128, SH), np.float32)      # host-transposed [feat, rows]
    for c in range(NB):
        lo, hi = c * nsh, min((c + 1) * nsh, N)
        d = np.zeros(SH, np.float32)
        d[: hi - lo] = dinv[lo:hi]
        dinv_cols[c] = d.reshape(NTL, 128).T
        x_sh[c, :, : hi - lo] = x[lo:hi].T

    return dict(
        N=N, nsh=nsh, SH=SH, NTL=NTL, NR=NR, QR=QR, tot=tot,
        nblocks=nblocks, NBM=NBM,
        P=tuple(map(tuple, P.tolist())),
        round_nblk=tuple(int(v) for v in round_nblk),
        sched=tuple(sched),
        gidx=gidx_w, oneh=oneh, dinv_cols=dinv_cols, x_sh=x_sh,
    )


# -------------------------------------------------------------- device side


def build_kernel(N, SH, NTL, NR, QR, tot, nblocks, NBM, P, round_nblk, sched,
                 D=128):
    nc = bacc.Bacc("TRN2", target_bir_lowering=False, debug=False,
                   num_devices=NB, num_swdge_queues=4)
    rg = [list(range(NB))]
    chunks512 = [(o, min(512, SH - o)) for o in range(0, SH, 512)]
    MTC = max(max(row) for row in P)               # max slots per call

    x_in = nc.dram_tensor("x", [D, SH], F32, kind="ExternalInput")
    gidx_in = nc.dram_tensor("gidx", [128, tot // 16], I16, kind="ExternalInput")
    oneh_in = nc.dram_tensor("oneh", [128, nblocks * 128], mybir.dt.float8e4,
                             kind="ExternalInput")
    dinv_in = nc.dram_tensor("dinv_cols", [128, NTL], F32, kind="ExternalInput")
    w_in = [nc.dram_tensor(f"w{l}", [D, D], F32, kind="ExternalInput")
            for l in range(2)]
    gam_in = [nc.dram_tensor(f"gamma{l}", [D, 1], F32, kind="ExternalInput")
              for l in range(2)]
    bet_in = [nc.dram_tensor(f"beta{l}", [D, 1], F32, kind="ExternalInput")
              for l in range(2)]
    a_in = [nc.dram_tensor(f"a{l}", [D, 1], F32, kind="ExternalInput")
            for l in range(2)]
    out_t = nc.dram_tensor("out", [SH, D], F32, kind="ExternalOutput")

    vloc = nc.dram_tensor("vloc", [SH, D], BF16)
    vfull = nc.dram_tensor("vfull", [NB * SH, D], BF16, addr_space="Shared")
    stats_in = nc.dram_tensor("stats_in", [D, 2], F32)
    stats_out = nc.dram_tensor("stats_out", [D, 2], F32, addr_space="Shared")

    out_r = out_t.ap().rearrange("(t p) f -> p t f", p=128)   # batched writes
    vloc_r = vloc.ap().rearrange("(t p) f -> p t f", p=128)

    with tile.TileContext(nc) as tc:
        with (
            tc.tile_pool(name="pers", bufs=1) as PE_,
            tc.tile_pool(name="act", bufs=1) as PA,
            tc.tile_pool(name="msg", bufs=3) as PM,
            tc.tile_pool(name="onehp", bufs=2) as PO,
            tc.tile_pool(name="work", bufs=3) as PW,
            tc.tile_pool(name="small", bufs=2) as PS,
            tc.tile_pool(name="psV", bufs=1, space="PSUM") as PV,
            tc.tile_pool(name="psA", bufs=5, space="PSUM") as PP,
            tc.tile_pool(name="psT", bufs=2, space="PSUM") as PT,
        ):
            ident = PE_.tile([128, 128], F32, tag="ident")
            make_identity(nc, ident[:])
            gidx_sb = PE_.tile([128, tot // 16], I16, tag="gidx")
            nc.sync.dma_start(gidx_sb[:], gidx_in.ap())

            dinv_sb = PE_.tile([128, NTL], F32, tag="dinv")
            nc.sync.dma_start(dinv_sb[:], dinv_in.ap())
            w_sb, gam_sb, bet_sb, a_sb = [], [], [], []
            for l in range(2):
                w_sb.append(PE_.tile([128, 128], F32, tag=f"w{l}", name=f"w{l}_sb"))
                nc.sync.dma_start(w_sb[l][:], w_in[l].ap())
                gam_sb.append(PE_.tile([128, 1], F32, tag=f"g{l}", name=f"g{l}_sb"))
                nc.sync.dma_start(gam_sb[l][:], gam_in[l].ap())
                bet_sb.append(PE_.tile([128, 1], F32, tag=f"b{l}", name=f"b{l}_sb"))
                nc.sync.dma_start(bet_sb[l][:], bet_in[l].ap())
                a_sb.append(PE_.tile([128, 1], F32, tag=f"a{l}", name=f"a{l}_sb"))
                nc.sync.dma_start(a_sb[l][:], a_in[l].ap())
            zero_sb = PE_.tile([128, 128], F32, tag="zero")
            nc.vector.memset(zero_sb[:], 0.0)
            eps_sb = PE_.tile([128, 1], F32, tag="eps")
            nc.vector.memset(eps_sb[:], EPS)

            actT = PA.tile([128, SH], F32, tag="actT")        # h [feat, rows]
            vself = PA.tile([128, NTL, 128], BF16, tag="vself")  # dinv^2*v rows

            # ---- load x (host-transposed) straight into actT
            for (o, cw) in chunks512:
                nc.sync.dma_start(actT[:, o : o + cw], x_in.ap()[:, o : o + cw])

            for l in range(2):
                # ---- v' = dinv * (h @ W) -> bf16 vloc + bf16 self term
                for (o, cw) in chunks512:
                    vp = PV.tile([128, 512], F32, tag="vp")
                    nc.tensor.matmul(out=vp[:, :cw], lhsT=w_sb[l][:],
                                     rhs=actT[:, o : o + cw],
                                     start=True, stop=True)
                    vt = PW.tile([128, 512], F32, tag="vt")
                    nc.vector.tensor_copy(vt[:, :cw], vp[:, :cw])
                    vvb = PW.tile([128, 4, 128], BF16, tag="vvb")
                    for s in range(0, cw, 128):
                        t = (o + s) // 128
                        tp = PT.tile([128, 128], F32, tag="tp")
                        nc.tensor.transpose(out=tp[:], in_=vt[:, s : s + 128],
                                            identity=ident[:])
                        nc.vector.tensor_scalar(
                            vvb[:, s // 128], tp[:], dinv_sb[:, t : t + 1],
                            None, op0=mybir.AluOpType.mult)
                        nc.vector.tensor_scalar(
                            vself[:, t], vvb[:, s // 128],
                            dinv_sb[:, t : t + 1], None,
                            op0=mybir.AluOpType.mult)
                    nc.sync.dma_start(
                        vloc_r[:, o // 128 : o // 128 + cw // 128, :],
                        vvb[:, : cw // 128, :])

                # ---- halo exchange
                nc.gpsimd.collective_compute(
                    "AllGather", mybir.AluOpType.bypass, replica_groups=rg,
                    ins=[vloc.ap().opt()], outs=[vfull.ap().opt()])

                # ---- gather + one-hot matmul segment sum
                sumc = PS.tile([128, NTL], F32, tag="sumc")
                sqc = PS.tile([128, NTL], F32, tag="sqc")
                soff = 0          # stream slot offset
                blk0 = 0          # oneh block offset
                for r in range(NR):
                    tiles = list(range(r * RT, min((r + 1) * RT, NTL)))
                    nblk_r = round_nblk[r]
                    ohl = PO.tile([128, NBM, 128], mybir.dt.float8e4,
                                  tag="ohl")
                    if nblk_r:
                        nc.sync.dma_start(
                            ohl[:, :nblk_r, :],
                            oneh_in.ap()[:, blk0 * 128 : (blk0 + nblk_r) * 128])
                    mts = []
                    for q in range(NQ):
                        sz = P[r][q]
                        mt = PM.tile([128, MTC // 128, 128], BF16,
                                     tag=f"mt{q}")
                        nc.gpsimd.dma_gather(
                            out_ap=mt[:, : sz // 128, :],
                            in_ap=vfull.ap()[q * QR : (q + 1) * QR, :],
                            idxs_ap=gidx_sb[:, soff // 16 : (soff + sz) // 16],
                            num_idxs=sz, num_idxs_reg=sz, elem_size=D,
                            single_packet=False, queue_num=q)
                        mts.append(mt)
                        soff += sz
                    for ti, t in enumerate(tiles):
                        lst = sched[r][ti]
                        if not lst:
                            sc = PW.tile([128, 128], F32, tag="sc")
                            nc.vector.tensor_copy(sc[:], vself[:, t])
                        else:
                            acc = PP.tile([128, 128], F32, tag="acc")
                            nb = len(lst)
                            for i, (q, k, rb) in enumerate(lst):
                                nc.tensor.matmul(
                                    out=acc[:], lhsT=ohl[:, rb, :],
                                    rhs=mts[q][:, k, :],
                                    start=(i == 0),
                                    stop=(i == nb - 1))
                            sc = PW.tile([128, 128], F32, tag="sc")
                            nc.vector.scalar_tensor_tensor(
                                out=sc[:], in0=acc[:],
                                scalar=dinv_sb[:, t : t + 1],
                                in1=vself[:, t],
                                op0=mybir.AluOpType.mult,
                                op1=mybir.AluOpType.add)
                        tp = PT.tile([128, 128], F32, tag="tp")
                        nc.tensor.transpose(out=tp[:], in_=sc[:],
                                            identity=ident[:])
                        nc.scalar.activation(
                            out=actT[:, 128 * t : 128 * (t + 1)], in_=tp[:],
                            func=mybir.ActivationFunctionType.Copy,
                            accum_out=sumc[:, t : t + 1])
                        sqd = PW.tile([128, 128], F32, tag="sqd")
                        nc.scalar.activation(
                            out=sqd[:], in_=tp[:],
                            func=mybir.ActivationFunctionType.Square,
                            bias=zero_sb[:, 0:1],
                            accum_out=sqc[:, t : t + 1])
                    blk0 += nblk_r

                # ---- BN stats (biased over real N rows; pad rows are 0)
                stats_sb = PS.tile([128, 2], F32, tag="stats")
                nc.vector.tensor_reduce(out=stats_sb[:, 0:1], in_=sumc[:],
                                        axis=mybir.AxisListType.X,
                                        op=mybir.AluOpType.add)
                nc.vector.tensor_reduce(out=stats_sb[:, 1:2], in_=sqc[:],
                                        axis=mybir.AxisListType.X,
                                        op=mybir.AluOpType.add)
                nc.sync.dma_start(stats_in.ap(), stats_sb[:])
                nc.gpsimd.collective_compute(
                    "AllReduce", mybir.AluOpType.add, replica_groups=rg,
                    ins=[stats_in.ap().opt()], outs=[stats_out.ap().opt()])
                stats2 = PS.tile([128, 2], F32, tag="stats2")
                nc.sync.dma_start(stats2[:], stats_out.ap())

                # ---- BN affine params
                mu = PS.tile([128, 1], F32, tag="mu")
                nc.vector.tensor_scalar(mu[:], stats2[:, 0:1], 1.0 / N, None,
                                        op0=mybir.AluOpType.mult)
                e2 = PS.tile([128, 1], F32, tag="e2")
                nc.vector.tensor_scalar(e2[:], stats2[:, 1:2], 1.0 / N, None,
                                        op0=mybir.AluOpType.mult)
                var = PS.tile([128, 1], F32, tag="var")
                nc.vector.scalar_tensor_tensor(
                    out=var[:], in0=mu[:], scalar=-1.0, in1=mu[:],
                    op0=mybir.AluOpType.mult, op1=mybir.AluOpType.mult)
                nc.vector.tensor_tensor(out=var[:], in0=e2[:], in1=var[:],
                                        op=mybir.AluOpType.add)
                sd = PS.tile([128, 1], F32, tag="sd")
                nc.scalar.activation(out=sd[:], in_=var[:],
                                     func=mybir.ActivationFunctionType.Sqrt,
                                     bias=eps_sb[:, 0:1])
                rinv = PS.tile([128, 1], F32, tag="rinv")
                nc.vector.reciprocal(rinv[:], sd[:])
                alpha = PS.tile([128, 1], F32, tag="alpha")
                nc.vector.tensor_tensor(out=alpha[:], in0=gam_sb[l][:],
                                        in1=rinv[:], op=mybir.AluOpType.mult)
                bias_p = PS.tile([128, 1], F32, tag="biasp")
                nc.vector.scalar_tensor_tensor(
                    out=bias_p[:], in0=alpha[:], scalar=-1.0, in1=mu[:],
                    op0=mybir.AluOpType.mult, op1=mybir.AluOpType.mult)
                nc.vector.tensor_tensor(out=bias_p[:], in0=bet_sb[l][:],
                                        in1=bias_p[:], op=mybir.AluOpType.add)
                nalpha = PS.tile([128, 1], F32, tag="nalpha")
                nc.vector.tensor_scalar(nalpha[:], alpha[:], -1.0, None,
                                        op0=mybir.AluOpType.mult)
                nbias = PS.tile([128, 1], F32, tag="nbias")
                nc.vector.tensor_scalar(nbias[:], bias_p[:], -1.0, None,
                                        op0=mybir.AluOpType.mult)
                na = PS.tile([128, 1], F32, tag="na")
                nc.vector.tensor_scalar(na[:], a_sb[l][:], -1.0, None,
                                        op0=mybir.AluOpType.mult)

                # ---- fused BN + PReLU: y = relu(z) - a*relu(-z)
                for (o, cw) in chunks512:
                    pos = PW.tile([128, 512], F32, tag="pos")
                    nc.scalar.activation(
                        out=pos[:, :cw], in_=actT[:, o : o + cw],
                        func=mybir.ActivationFunctionType.Relu,
                        bias=bias_p[:, :1], scale=alpha[:, :1])
                    neg = PW.tile([128, 512], F32, tag="neg")
                    nc.scalar.activation(
                        out=neg[:, :cw], in_=actT[:, o : o + cw],
                        func=mybir.ActivationFunctionType.Relu,
                        bias=nbias[:, :1], scale=nalpha[:, :1])
                    nc.vector.scalar_tensor_tensor(
                        out=actT[:, o : o + cw], in0=neg[:, :cw],
                        scalar=na[:, :1], in1=pos[:, :cw],
                        op0=mybir.AluOpType.mult, op1=mybir.AluOpType.add)

            # ---- write h2 back as [rows, feat], batched 4 tiles per DMA
            for (o, cw) in chunks512:
                otb = PW.tile([128, 4, 128], F32, tag="otb")
                for s in range(0, cw, 128):
                    t = (o + s) // 128
                    tp = PT.tile([128, 128], F32, tag="tp")
                    nc.tensor.transpose(out=tp[:],
                                        in_=actT[:, 128 * t : 128 * (t + 1)],
                                        identity=ident[:])
                    nc.vector.tensor_copy(otb[:, s // 128], tp[:])
                nc.sync.dma_start(out_r[:, o // 128 : o // 128 + cw // 128, :],
                                  otb[:, : cw // 128, :])

    nc.compile()
    return nc


# ------------------------------------------------------------------- driver

_CACHE: dict = {}


def _get_compiled(pre):
    key = (pre["N"], pre["SH"], pre["tot"], pre["nblocks"], pre["P"],
           pre["sched"])
    if key not in _CACHE:
        import os
        nc = build_kernel(pre["N"], pre["SH"], pre["NTL"], pre["NR"],
                          pre["QR"], pre["tot"], pre["nblocks"], pre["NBM"],
                          pre["P"], pre["round_nblk"], pre["sched"])
        if not os.environ.get("KN_SIM"):
            nc.m = get_hw_module(nc.m)
        _CACHE[key] = nc
    return _CACHE[key]


def make_in_maps(pre, w0, gamma0, beta0, a0, w1, gamma1, beta1, a1):
    def col(v):
        return np.ascontiguousarray(np.asarray(v, np.float32).reshape(-1, 1))

    def rep(v):
        return np.full((128, 1), np.float32(np.asarray(v).reshape(-1)[0]),
                       np.float32)

    maps = []
    for c in range(NB):
        maps.append({
            "x": pre["x_sh"][c],
            "gidx": pre["gidx"][c],
            "oneh": pre["oneh"][c],
            "dinv_cols": pre["dinv_cols"][c],
            "w0": np.ascontiguousarray(np.asarray(w0, np.float32)),
            "w1": np.ascontiguousarray(np.asarray(w1, np.float32)),
            "gamma0": col(gamma0), "beta0": col(beta0), "a0": rep(a0),
            "gamma1": col(gamma1), "beta1": col(beta1), "a1": rep(a1),
        })
    return maps


def kernel(x, edge_index, w0, b0, gamma0, beta0, a0,
           w1, b1, gamma1, beta1, a1, _trace=False):
    x = np.asarray(x, np.float32)
    edge_index = np.asarray(edge_index, np.int64)
    pre = preprocess(x, edge_index)
    nc = _get_compiled(pre)
    in_maps = make_in_maps(pre, w0, gamma0, beta0, a0, w1, gamma1, beta1, a1)
    res = bass_utils.run_bass_kernel_spmd(
        nc, in_maps, core_ids=list(range(NB)), trace=_trace)
    nsh, N = pre["nsh"], pre["N"]
    out = np.concatenate([res.results[c]["out"][:nsh] for c in range(NB)],
                         axis=0)[:N]
    if _trace:
        kernel.last_results = res
    return np.ascontiguousarray(out)
